# revision 9
# baseline (speedup 1.0000x reference)
"""Trainium2 Bass kernel for nn_DREAMAcousticNL (2-layer liquid-RNN over T=1000).

Strategy
--------
Key algebraic facts about the reference:
  * cell0's recurrent state h0 is dead code: `base_eff` (the signal fed to
    cell1 and to the output) depends only on x_t.  So only cell1's scan is
    sequential.
  * Everything that depends only on the inputs is precomputed as batched
    matmuls (phase A): be0 = clip_norm(x) @ B0.T, its norm xs1, be0' =
    be0/xs1, be1 = clip(be0') @ B1.T, plus the be1-part of the output head.
  * Per scan step t only:  m1 = h @ C1.T ; th = tanh(m1) ; err' = be0'-th ;
    ss = |err'|^2 ; s = sigmoid(P3(ss)) with P3 a cubic fitted to
    (min(sqrt(ss),4)-tau)/gamma on the presimulated ss range (tanh and
    sigmoid share one ACT table, so no table thrash) ; u = err' @ W1.T ;
    ih = 0.2 h + 0.6 be1 + (0.2 s xs1) u ; h' = h (1-g) + tanh(ih) g with
    g = s*sig(a1).
  * Head: y = h_seq @ head_w[:,:512].T + be1 @ head_w[:,512:].T + head_b.

Wall-clock structure (the graded metric): the bass_exec path recompiles
BIR->NEFF per run and re-uploads all inputs through the axon relay, so
program SIZE and WIRE BYTES/ARRAY COUNT dominate, not device time (~50ms).
Hence:
  * The scan runs as a tc.For_i HARDWARE loop (UNROLL steps/iteration) with
    ds() dynamic column offsets into SBUF history tensors (~1k instrs
    total vs ~55k fully unrolled -> seconds of neuronx-cc per run saved).
  * All large inputs ship as ONE bf16 pack + ONE small f32 pack per core
    (upload latency is per-array x per-shard); weights are converted to
    f32 in SBUF right after DMA, so compute precision is unchanged except
    for the bf16 rounding of the weights/feats themselves.
  * The surprise-poly coefficients and clamp range are RUNTIME inputs, so
    build_program is input-independent and the numpy presimulation that
    produces them runs on a thread concurrently with build+compile.
  * The output ships bf16 (halves the donated-zero upload + fetch).

Distribution: data-parallel over batch, B=16 -> 8 cores x 2 rows (SPMD).

Layout ("T-layout"): features on partitions.  Per-core tensors are
(128, 4, 2*T): partition p + chunk kc hold feature kc*128+p, free column
t*2+b.  Per-step matmuls use 128x128 stationary weight blocks
(lhsT = W.T block) with the (128,2) state slice as the moving operand, so
outputs stay feature-on-partition.  Norm reductions across partitions use a
ones-column matmul; per-row scalars are broadcast back to 128 partitions
with a rank-1 outer-product matmul.
"""

import os
import threading

import numpy as np
import ml_dtypes

B, T, MEL, HID, NCLS = 16, 1000, 80, 512, 64
NCORES = 8
BLOC = B // NCORES          # 2 batch rows per core
NT = T * BLOC               # free columns of history tensors
KC = HID // 128             # 4 feature chunks
NCH = 4                     # phase-A column chunks

# bf16 pack offsets (elements)
SZ_FEATS = MEL * NT
SZ_W = KC * 128 * HID
SZ_B0 = MEL * HID
SZ_WH = KC * 128 * NCLS
OF_FEATS = 0
OF_C1 = OF_FEATS + SZ_FEATS
OF_W1 = OF_C1 + SZ_W
OF_B1 = OF_W1 + SZ_W
OF_B0 = OF_B1 + SZ_W
OF_WH1 = OF_B0 + SZ_B0
OF_WH2 = OF_WH1 + SZ_WH
NB = OF_WH2 + SZ_WH
# f32 pack offsets
OFS_HEADB = 0
OFS_SIGA = OFS_HEADB + NCLS
OFS_COEF = OFS_SIGA + 128 * KC * BLOC
NS = OFS_COEF + 8

_LAST_RUN = {}
_TP = lambda msg: None   # timing probe, set by kernel() when KERNEL_TIMING=1


# ---------------------------------------------------------------- host math
def _np_phase_a(inputs):
    f32 = np.float32
    feats = inputs["feats"].astype(f32)
    ss0 = np.einsum("btm,btm->bt", feats, feats)
    xs0 = np.maximum(np.sqrt(ss0), 1e-6)[..., None].astype(f32)
    xn0 = np.clip(feats * (f32(1.0) / xs0), -1, 1).astype(f32)
    be0 = (xn0 @ inputs["B0"].astype(f32).T).astype(f32)
    ss1 = np.einsum("btm,btm->bt", be0, be0)
    xs1 = np.maximum(np.sqrt(ss1), 1e-6)[..., None].astype(f32)
    be0p = (be0 * (f32(1.0) / xs1)).astype(f32)
    xn1 = np.clip(be0p, -1, 1).astype(f32)
    be1 = (xn1 @ inputs["B1"].astype(f32).T).astype(f32)
    return be0p, xs1[..., 0], be1


def _np_ss_range(inputs, t_run):
    """Presimulate the scan in numpy to find the range of ss = |err'|^2."""
    f32 = np.float32
    be0p, xs1, be1 = _np_phase_a(inputs)
    C1 = inputs["C1"].astype(f32)
    W1 = inputs["W1"].astype(f32)
    a1 = inputs["a1"].astype(f32)
    tau = float(inputs["tau01"][0])
    gam = float(inputs["gamma1"][0])
    siga = (1 / (1 + np.exp(-a1))).astype(f32)
    h = np.zeros((B, HID), f32)
    lo, hi = np.inf, -np.inf
    for t in range(t_run):
        m1 = h @ C1.T
        err = be0p[:, t, :] - np.tanh(m1)
        ss = np.einsum("bh,bh->b", err, err)
        lo = min(lo, ss.min()); hi = max(hi, ss.max())
        rel = np.minimum(np.sqrt(ss), 4.0)
        s = 1 / (1 + np.exp(-(rel - tau) / gam))
        u = err @ W1.T
        ih = 0.2 * h + 0.6 * be1[:, t, :] + (0.2 * s * xs1[:, t])[:, None] * u
        h = h + (s[:, None] * siga[None, :]) * (np.tanh(ih) - h)
    return float(lo), float(hi)


def _fit_sarg_poly(tau, gam, lo, hi):
    """Cubic fit of (min(sqrt(x),4)-tau)/gam on [lo,hi]; coeffs highest-first."""
    xg = np.linspace(lo, hi, 2001)
    tgt = (np.minimum(np.sqrt(xg), 4.0) - tau) / gam
    ch = np.polynomial.chebyshev.Chebyshev.fit(xg, tgt, 3)
    co = np.polynomial.chebyshev.cheb2poly(ch.convert().coef)
    err = np.max(np.abs(np.polyval(co[::-1], xg) - tgt))
    return [float(c) for c in co[::-1]], float(err)


# ------------------------------------------------------------- bass program
def build_program(t_run, num_devices=NCORES):
    import concourse.bacc as bacc
    import concourse.bass as bass
    import concourse.mybir as mybir
    import concourse.tile as tile

    ds = bass.ds
    dt = mybir.dt
    f32 = dt.float32
    bf16 = dt.bfloat16
    HDT = bf16
    AF = mybir.ActivationFunctionType
    OP = mybir.AluOpType
    ntr = t_run * BLOC
    chw = ntr // NCH
    UNROLL = int(os.environ.get("KERNEL_UNROLL", "4"))
    STEP = UNROLL * BLOC
    assert ntr % NCH == 0 and ntr % STEP == 0

    nc = bacc.Bacc("TRN2", target_bir_lowering=False, debug=False,
                   num_devices=num_devices)

    packb = nc.dram_tensor("packb", (NB,), bf16, kind="ExternalInput").ap()
    packs = nc.dram_tensor("packs", (NS,), f32, kind="ExternalInput").ap()
    yt = nc.dram_tensor("yt", (NCLS, NT), bf16, kind="ExternalOutput").ap()

    def seg_b(ofs, sz):
        return packb[ofs:ofs + sz]

    with tile.TileContext(nc) as tc:
        with (
            tc.tile_pool(name="const", bufs=1) as cpool,
            tc.tile_pool(name="hist", bufs=1) as hpool,
            tc.tile_pool(name="pha", bufs=2) as apool,
            tc.tile_pool(name="scan", bufs=2) as spool,
            tc.tile_pool(name="pp", bufs=2, space="PSUM") as pp,
        ):
            # ---- unpack + bf16->f32 convert of weights/feats
            sb_feats = cpool.tile([MEL, ntr], f32)
            sb_c1 = cpool.tile([128, KC, HID], f32)
            sb_w1 = cpool.tile([128, KC, HID], f32)
            sb_b0 = cpool.tile([MEL, HID], f32)
            sb_b1 = cpool.tile([128, KC, HID], f32)
            sb_wh1 = cpool.tile([128, KC, NCLS], f32)
            sb_wh2 = cpool.tile([128, KC, NCLS], f32)
            sb_headb = cpool.tile([NCLS, 1], f32)
            sb_siga = cpool.tile([128, KC, BLOC], f32)
            sb_cf = cpool.tile([1, 8], f32)
            sb_ones = cpool.tile([128, 1], f32)       # column of ones
            sb_onesT = cpool.tile([1, 128], f32)      # row of ones

            def load_cvt(dst, ofs, sz, pat, tag, **kw):
                stg = apool.tile(list(dst.shape), bf16, tag=tag)
                nc.sync.dma_start(stg[:], seg_b(ofs, sz).rearrange(pat, **kw))
                nc.vector.tensor_copy(dst[:], stg[:])

            load_cvt(sb_feats, OF_FEATS, SZ_FEATS, "(m n) -> m n", "lf",
                     m=MEL)
            load_cvt(sb_c1, OF_C1, SZ_W, "(k p n) -> p k n", "l0", k=KC,
                     p=128)
            load_cvt(sb_w1, OF_W1, SZ_W, "(k p n) -> p k n", "l1", k=KC,
                     p=128)
            load_cvt(sb_b1, OF_B1, SZ_W, "(k p n) -> p k n", "l0", k=KC,
                     p=128)
            load_cvt(sb_b0, OF_B0, SZ_B0, "(m n) -> m n", "l1", m=MEL)
            load_cvt(sb_wh1, OF_WH1, SZ_WH, "(k p n) -> p k n", "lw",
                     k=KC, p=128)
            load_cvt(sb_wh2, OF_WH2, SZ_WH, "(k p n) -> p k n", "lw",
                     k=KC, p=128)
            nc.sync.dma_start(
                sb_headb[:],
                packs[OFS_HEADB:OFS_HEADB + NCLS].rearrange("(m n) -> m n",
                                                            m=NCLS))
            nc.sync.dma_start(
                sb_siga[:],
                packs[OFS_SIGA:OFS_SIGA + 128 * KC * BLOC].rearrange(
                    "(p k b) -> p k b", p=128, k=KC))
            nc.sync.dma_start(
                sb_cf[:],
                packs[OFS_COEF:OFS_COEF + 8].rearrange("(m n) -> m n", m=1))
            nc.vector.memset(sb_ones[:], 1.0)
            nc.vector.memset(sb_onesT[:], 1.0)

            # ---- persistent per-core state
            sb_be0f = hpool.tile([128, KC, ntr], f32)   # be0 then (in-place) xn1
            sb_be0p = hpool.tile([128, KC, ntr], HDT)   # be0/xs1 history
            sb_be1s = hpool.tile([128, KC, ntr], HDT)   # 0.6*be1 history
            # h history, 1 zero slot in front: column o = t*BLOC is h BEFORE
            # step t; the scan writes h_t at o+BLOC
            sb_hs = hpool.tile([128, KC, ntr + BLOC], f32)
            sb_xs1s = hpool.tile([1, ntr], f32)         # 0.2*xs1
            sb_yt = hpool.tile([NCLS, ntr], f32)

            # ================= phase A =================
            for ch in range(NCH):
                R = slice(ch * chw, (ch + 1) * chw)
                sq = apool.tile([128, chw], f32, tag="sqA")
                p_ss = pp.tile([1, chw], f32, tag="p2")
                p_bc = pp.tile([128, chw], f32, tag="p3")
                rowA = apool.tile([1, chw], f32, tag="rowA")
                rowB = apool.tile([1, chw], f32, tag="rowB")

                # |x|^2 over 80 input dims
                nc.vector.tensor_tensor(sq[:MEL, :], sb_feats[:, R],
                                        sb_feats[:, R], OP.mult)
                nc.tensor.matmul(p_ss[:], sb_ones[:MEL, :], sq[:MEL, :],
                                 start=True, stop=True)
                nc.scalar.activation(rowA[:], p_ss[:], AF.Sqrt)
                nc.vector.tensor_scalar_max(rowA[:], rowA[:], 1e-6)
                nc.vector.reciprocal(rowB[:], rowA[:])
                nc.tensor.matmul(p_bc[:MEL, :], sb_onesT[:, :MEL], rowB[:],
                                 start=True, stop=True)
                # xn0 = clip(x/|x|) in place
                nc.vector.tensor_tensor(sb_feats[:, R], sb_feats[:, R],
                                        p_bc[:MEL, :], OP.mult)
                nc.vector.tensor_scalar(sb_feats[:, R], sb_feats[:, R],
                                        -1.0, 1.0, OP.max, OP.min)
                # be0 = xn0 @ B0.T
                for mc in range(KC):
                    p_be = pp.tile([128, chw], f32, tag="p0")
                    nc.tensor.matmul(p_be[:], sb_b0[:, mc * 128:(mc + 1) * 128],
                                     sb_feats[:, R], start=True, stop=True)
                    nc.vector.tensor_copy(sb_be0f[:, mc, R], p_be[:])
                # |be0|^2 over 512
                for kc in range(KC):
                    nc.vector.tensor_tensor(sq[:], sb_be0f[:, kc, R],
                                            sb_be0f[:, kc, R], OP.mult)
                    nc.tensor.matmul(p_ss[:], sb_ones[:], sq[:],
                                     start=(kc == 0), stop=(kc == KC - 1))
                nc.scalar.activation(rowA[:], p_ss[:], AF.Sqrt)
                nc.vector.tensor_scalar_max(rowA[:], rowA[:], 1e-6)
                nc.vector.tensor_scalar_mul(sb_xs1s[:, R], rowA[:], 0.2)
                nc.vector.reciprocal(rowB[:], rowA[:])
                nc.tensor.matmul(p_bc[:], sb_onesT[:], rowB[:],
                                 start=True, stop=True)
                for kc in range(KC):
                    # be0' = be0/xs1 (bf16 history), xn1 = clip(be0') in place
                    nc.vector.tensor_tensor(sb_be0p[:, kc, R], sb_be0f[:, kc, R],
                                            p_bc[:], OP.mult)
                    nc.vector.tensor_tensor(sb_be0f[:, kc, R], sb_be0f[:, kc, R],
                                            p_bc[:], OP.mult)
                    nc.vector.tensor_scalar(sb_be0f[:, kc, R], sb_be0f[:, kc, R],
                                            -1.0, 1.0, OP.max, OP.min)
                # be1 = xn1 @ B1.T ; also y2 = Wh2 @ be1T accumulated
                p_y2 = pp.tile([NCLS, chw], f32, tag="p1")
                tmp_be1 = apool.tile([128, chw], f32, tag="tbe1A")
                for mc in range(KC):
                    p_be1 = pp.tile([128, chw], f32, tag="p0")
                    for kc in range(KC):
                        nc.tensor.matmul(
                            p_be1[:],
                            sb_b1[:, kc, mc * 128:(mc + 1) * 128],
                            sb_be0f[:, kc, R],
                            start=(kc == 0), stop=(kc == KC - 1))
                    nc.vector.tensor_scalar_mul(sb_be1s[:, mc, R], p_be1[:], 0.6)
                    nc.scalar.copy(tmp_be1[:], p_be1[:])
                    nc.tensor.matmul(p_y2[:], sb_wh2[:, mc, :], tmp_be1[:],
                                     start=(mc == 0), stop=(mc == KC - 1))
                nc.vector.tensor_copy(sb_yt[:, R], p_y2[:])

            # zero h slot 0
            nc.vector.memset(sb_hs[:, :, 0:BLOC], 0.0)

            # scheduler fence: keep phase-A ACT (sqrt set) strictly before the
            # scan's tanh/sigmoid stream to avoid activation-table thrash
            tc.no_sync_barrier()

            # ================= phase B: the scan (hardware loop) ============
            with tc.For_i(0, ntr, STEP) as iv:
                for k in range(UNROLL):
                    o = iv + k * BLOC

                    pm1 = pp.tile([128, KC * BLOC], f32, tag="p0")
                    pu = pp.tile([128, KC * BLOC], f32, tag="p1")
                    pss = pp.tile([1, KC * BLOC], f32, tag="p2")
                    pbc = pp.tile([128, 2 * BLOC], f32, tag="p3")
                    th = spool.tile([128, KC, BLOC], HDT, tag="th")
                    err = spool.tile([128, KC, BLOC], f32, tag="err")
                    sqt = spool.tile([128, KC * BLOC], f32, tag="sqt")
                    ssb = spool.tile([1, BLOC], f32, tag="ssb")
                    acc = spool.tile([1, BLOC], f32, tag="acc")
                    accB = spool.tile([1, BLOC], f32, tag="accB")
                    row4 = spool.tile([1, 2 * BLOC], f32, tag="row4")
                    gt = spool.tile([128, KC, BLOC], f32, tag="gt")
                    gtm = spool.tile([128, KC, BLOC], f32, tag="gtm")
                    p1 = spool.tile([128, KC, BLOC], f32, tag="p1s")
                    vt = spool.tile([128, KC, BLOC], f32, tag="vt")
                    wt = spool.tile([128, KC, BLOC], f32, tag="wt")
                    ih = spool.tile([128, KC, BLOC], f32, tag="ih")
                    th2 = spool.tile([128, KC, BLOC], f32, tag="th2")
                    q1 = spool.tile([128, KC, BLOC], f32, tag="q1")

                    # m1.T = C1 @ h.T (16 blocks, moving operand = h slice)
                    for mc in range(KC):
                        for kc in range(KC):
                            nc.tensor.matmul(
                                pm1[:, mc * BLOC:(mc + 1) * BLOC],
                                sb_c1[:, kc, mc * 128:(mc + 1) * 128],
                                sb_hs[:, kc, ds(o, BLOC)],
                                start=(kc == 0), stop=(kc == KC - 1))
                    nc.scalar.activation(th[:], pm1[:], AF.Tanh)
                    nc.vector.tensor_tensor(err[:], sb_be0p[:, :, ds(o, BLOC)],
                                            th[:], OP.subtract)
                    nc.vector.tensor_tensor(sqt[:], err[:], err[:], OP.mult)
                    nc.tensor.matmul(pss[:], sb_ones[:], sqt[:],
                                     start=True, stop=True)
                    # ss per row: sum the 4 chunk partials (cols kc-major)
                    nc.vector.tensor_reduce(
                        ssb[:], pss.rearrange("p (k b) -> p b k", k=KC),
                        mybir.AxisListType.X, OP.add)
                    # s = sigmoid(P3(clamp(ss))), P3 ~ (min(sqrt,4)-tau)/gam
                    nc.vector.tensor_scalar(ssb[:], ssb[:], sb_cf[:, 0:1],
                                            sb_cf[:, 1:2], OP.max, OP.min)
                    nc.vector.tensor_scalar(acc[:], ssb[:], sb_cf[:, 2:3],
                                            sb_cf[:, 3:4], OP.mult, OP.add)
                    nc.vector.tensor_tensor(accB[:], acc[:], ssb[:], OP.mult)
                    nc.vector.tensor_scalar_add(accB[:], accB[:], sb_cf[:, 4:5])
                    nc.vector.tensor_tensor(acc[:], accB[:], ssb[:], OP.mult)
                    nc.scalar.activation(row4[:, 0:BLOC], acc[:], AF.Sigmoid,
                                         bias=sb_cf[:, 5:6])
                    # c = 0.2*s*xs1
                    nc.vector.tensor_tensor(row4[:, BLOC:2 * BLOC],
                                            row4[:, 0:BLOC],
                                            sb_xs1s[:, ds(o, BLOC)], OP.mult)
                    # u.T = W1 @ err'.T
                    for mc in range(KC):
                        for kc in range(KC):
                            nc.tensor.matmul(
                                pu[:, mc * BLOC:(mc + 1) * BLOC],
                                sb_w1[:, kc, mc * 128:(mc + 1) * 128],
                                err[:, kc, :],
                                start=(kc == 0), stop=(kc == KC - 1))
                    # broadcast [s0,s1,c0,c1] to all partitions
                    nc.tensor.matmul(pbc[:], sb_onesT[:], row4[:],
                                     start=True, stop=True)
                    puv = pu.rearrange("p (k b) -> p k b", k=KC)
                    # g = s*sig(a1); gm = 1-g; both via stride-0 kc-broadcast
                    nc.vector.tensor_tensor(
                        gt[:], sb_siga[:],
                        pbc[:, None, 0:BLOC].broadcast_to([128, KC, BLOC]),
                        OP.mult)
                    nc.vector.tensor_scalar(gtm[:], gt[:], -1.0, 1.0,
                                            OP.mult, OP.add)
                    nc.vector.tensor_tensor(p1[:], sb_hs[:, :, ds(o, BLOC)],
                                            gtm[:], OP.mult)
                    # vt = c*u: u is in PSUM, so the c broadcast must come
                    # from SBUF (TensorTensor reads at most one PSUM input)
                    sbc2 = spool.tile([128, BLOC], f32, tag="sbc2")
                    nc.vector.tensor_copy(sbc2[:], pbc[:, BLOC:2 * BLOC])
                    nc.vector.tensor_tensor(
                        vt[:], puv,
                        sbc2[:, None, :].broadcast_to([128, KC, BLOC]),
                        OP.mult)
                    # ih = 0.2 h + 0.6 be1 + c*u
                    nc.vector.scalar_tensor_tensor(
                        wt[:], sb_hs[:, :, ds(o, BLOC)], 0.2,
                        sb_be1s[:, :, ds(o, BLOC)], OP.mult, OP.add)
                    nc.vector.tensor_tensor(ih[:], wt[:], vt[:], OP.add)
                    nc.scalar.activation(th2[:], ih[:], AF.Tanh)
                    # h' = h*(1-g) + tanh(ih)*g
                    nc.vector.tensor_tensor(q1[:], th2[:], gt[:], OP.mult)
                    nc.vector.tensor_tensor(sb_hs[:, :, ds(o + BLOC, BLOC)],
                                            p1[:], q1[:], OP.add)

            tc.no_sync_barrier()

            # ================= phase C: head =================
            for ch in range(NCH):
                R = slice(ch * chw, (ch + 1) * chw)
                Rh = slice(BLOC + ch * chw, BLOC + (ch + 1) * chw)
                p_y1 = pp.tile([NCLS, chw], f32, tag="p1")
                ytb = apool.tile([NCLS, chw], bf16, tag="ytb")
                for kc in range(KC):
                    nc.tensor.matmul(p_y1[:], sb_wh1[:, kc, :],
                                     sb_hs[:, kc, Rh],
                                     start=(kc == 0), stop=(kc == KC - 1))
                nc.vector.tensor_tensor(sb_yt[:, R], sb_yt[:, R], p_y1[:],
                                        OP.add)
                nc.vector.tensor_scalar(ytb[:], sb_yt[:, R],
                                        sb_headb[:], None, OP.add)
                nc.sync.dma_start(yt[:, R], ytb[:])

    nc.compile()
    return nc


# ------------------------------------------------------------- input prep
def _prep_inputs(inputs):
    """Build the per-core packed input maps (host-side layout only)."""
    f32c = lambda k: np.ascontiguousarray(inputs[k], dtype=np.float32)
    bf16 = ml_dtypes.bfloat16

    C1T = np.ascontiguousarray(f32c("C1").T)            # (512,512) [k, m]
    W1T = np.ascontiguousarray(f32c("W1").T)
    B1T = np.ascontiguousarray(f32c("B1").T)
    B0T = np.ascontiguousarray(f32c("B0").T)            # (80,512)
    head_w = f32c("head_w")
    Wh1T = np.ascontiguousarray(head_w[:, :HID].T)      # (512,64)
    Wh2T = np.ascontiguousarray(head_w[:, HID:].T)
    a1 = f32c("a1")
    siga = (1.0 / (1.0 + np.exp(-a1))).astype(np.float32)   # (512,)
    # (128, KC, BLOC): siga8[p, kc, b] = siga[kc*128+p]
    siga8 = np.repeat(siga.reshape(KC, 128).T[:, :, None], BLOC, axis=2)

    wseg = np.empty(NB - SZ_FEATS, bf16)

    def put(ofs, arr):
        fl = np.asarray(arr, np.float32).reshape(-1)
        wseg[ofs - SZ_FEATS:ofs - SZ_FEATS + fl.size] = fl.astype(bf16)

    put(OF_C1, C1T)       # flat (k p n): C1T.reshape(KC,128,512) row-major
    put(OF_W1, W1T)
    put(OF_B1, B1T)
    put(OF_B0, B0T)
    put(OF_WH1, Wh1T)
    put(OF_WH2, Wh2T)

    packs = np.zeros(NS, np.float32)
    packs[OFS_HEADB:OFS_HEADB + NCLS] = f32c("head_b")
    packs[OFS_SIGA:OFS_SIGA + 128 * KC * BLOC] = siga8.reshape(-1)
    # packs[OFS_COEF:] filled later (after the presim joins)

    feats = f32c("feats")                                # (16,1000,80)
    in_maps = []
    for c in range(NCORES):
        fl = feats[c * BLOC:(c + 1) * BLOC]              # (2,1000,80)
        # featsT[m, t*2+b] = feats[b, t, m]
        ftT = fl.transpose(2, 1, 0).reshape(-1).astype(bf16)
        packb = np.concatenate([ftT, wseg])
        in_maps.append({"packb": packb, "packs": packs})
    return in_maps


# ------------------------------------------------------------- fast runner
def _run_fast(nc, in_maps, pre):
    """shard_map runner for the prebuilt Bass module.  Vs the stock
    bass_utils path it (a) consumes inputs ALREADY device_put asynchronously
    during build/compile (`pre`), so the 17MB upload overlaps the host-side
    compile instead of serializing after it, and (b) gathers the sharded
    output once instead of once per core.  Any failure falls back to
    bass_utils.run_bass_kernel_spmd in kernel()."""
    import jax
    import concourse.mybir as mybir
    from concourse.bass2jax import (_bass_exec_p, install_neuronx_cc_hook,
                                    partition_id_tensor)
    from jax.sharding import Mesh, PartitionSpec
    try:
        from jax.experimental.shard_map import shard_map
    except ImportError:
        shard_map = jax.shard_map

    install_neuronx_cc_hook()
    assert nc.dbg_addr is None
    n_cores = len(in_maps)

    in_names, out_names, out_avals, zero_shapes = [], [], [], []
    for alloc in nc.m.functions[0].allocations:
        if not isinstance(alloc, mybir.MemoryLocationSet):
            continue
        name = alloc.memorylocations[0].name
        if alloc.kind == "ExternalInput":
            in_names.append(name)
        elif alloc.kind == "ExternalOutput":
            shape = tuple(alloc.tensor_shape)
            dtype = mybir.dt.np(alloc.dtype)
            out_names.append(name)
            out_avals.append(jax.core.ShapedArray(shape, dtype))
            zero_shapes.append((shape, dtype))
    partition_name = (nc.partition_id_tensor.name
                      if nc.partition_id_tensor else None)
    if partition_name is not None:
        in_names.remove(partition_name)
    n_params = len(in_names)
    n_outs = len(out_names)
    all_names = in_names + out_names + (
        [partition_name] if partition_name else [])
    donate = tuple(range(n_params, n_params + n_outs))

    def _body(*args):
        operands = list(args)
        if partition_name is not None:
            operands.append(partition_id_tensor())
        outs = _bass_exec_p.bind(
            *operands, out_avals=tuple(out_avals), in_names=tuple(all_names),
            out_names=tuple(out_names), lowering_input_output_aliases=(),
            sim_require_finite=True, sim_require_nnan=True, nc=nc)
        return tuple(outs)

    devices = jax.devices()[:n_cores]
    mesh = Mesh(np.asarray(devices), ("core",))
    in_specs = (PartitionSpec("core"),) * (n_params + n_outs)
    out_specs = (PartitionSpec("core"),) * n_outs
    sharded = jax.jit(
        shard_map(_body, mesh=mesh, in_specs=in_specs, out_specs=out_specs,
                  check_rep=False),
        donate_argnums=donate, keep_unused=True)

    pre = pre or {}
    args = []
    for name in in_names:
        if name in pre:
            args.append(pre[name])
        else:
            args.append(np.concatenate(
                [np.asarray(in_maps[c][name]) for c in range(n_cores)],
                axis=0))
    for name, (shape, dtype) in zip(out_names, zero_shapes):
        zkey = f"__zeros_{name}__"
        gshape = (n_cores * shape[0], *shape[1:])
        if (zkey in pre and tuple(pre[zkey].shape) == gshape
                and pre[zkey].dtype == dtype):
            args.append(pre[zkey])
        else:
            args.append(np.zeros(gshape, dtype))
    _TP("run_fast: args ready")
    compiled = sharded.lower(*args).compile()
    _TP("run_fast: jit lower+compile done")
    import time as _time
    from concurrent.futures import ThreadPoolExecutor
    _t0 = _time.perf_counter()
    out_arrs = compiled(*args)
    _TP("run_fast: dispatch returned")
    jax.block_until_ready(out_arrs)
    _TP("run_fast: device done")
    res = {}
    for i, name in enumerate(out_names):
        arr = out_arrs[i]
        try:
            # fetch the 8 per-core shards concurrently: the download is
            # relay-RTT-bound, so serial per-shard pulls cost 8 round trips
            shards = sorted(arr.addressable_shards,
                            key=lambda s: s.index[0].start or 0)
            with ThreadPoolExecutor(len(shards)) as ex:
                parts = list(ex.map(lambda s: np.asarray(s.data), shards))
            res[name] = np.concatenate(parts, axis=0)
        except Exception:
            res[name] = np.asarray(arr)
    # dispatch→fetch wall: device execution + result download (upper bound
    # on HW exec; excludes host-side jit/NEFF compilation, which is not
    # execution).  Recorded for kernel() to surface as exec_time_ns.
    res["__exec_wall_s__"] = _time.perf_counter() - _t0
    _TP("run_fast: outputs fetched")
    return res


def kernel(**inputs):
    import time as _time
    import sys as _sys

    _tstart = _time.perf_counter()
    _dbg = os.environ.get("KERNEL_TIMING")

    def _tp(msg):
        if _dbg:
            print(f"[ktime {_time.perf_counter() - _tstart:8.2f}s] {msg}",
                  file=_sys.stderr, flush=True)

    # inputs may arrive as jax arrays; all host math below assumes numpy
    inputs = {k: np.asarray(v) for k, v in inputs.items()}
    global _TP
    _TP = _tp
    _tp("inputs to numpy")

    t_run = int(os.environ.get("KERNEL_T", T))

    # presimulation (for the surprise-poly fit range) runs concurrently
    # with build_program: its results are runtime inputs, not constants.
    # A second thread forces jax/PJRT (axon) init, packs the inputs, and
    # starts the async upload of the big per-core pack + donated output
    # zeros, so both overlap the bass build + jit compile.
    presim = {}
    def _presim():
        presim["r"] = _np_ss_range(inputs, t_run)
        _tp("presim done")
    th = threading.Thread(target=_presim)
    th.start()
    warm = {}
    def _warm():
        import jax
        devs = jax.devices()
        _tp("jax.devices ready")
        warm["maps"] = _prep_inputs(inputs)
        _tp("prep_inputs done")
        try:
            from jax.sharding import Mesh, PartitionSpec, NamedSharding
            mesh = Mesh(np.asarray(devs[:NCORES]), ("core",))
            sh = NamedSharding(mesh, PartitionSpec("core"))
            cb = np.concatenate([m["packb"] for m in warm["maps"]])
            z = np.zeros((NCORES * NCLS, NT), ml_dtypes.bfloat16)
            warm["pre"] = {"packb": jax.device_put(cb, sh),
                           "__zeros_yt__": jax.device_put(z, sh)}
            _tp("device_put issued")
        except Exception:
            warm["pre"] = None
    tw = threading.Thread(target=_warm)
    tw.start()

    nc = build_program(t_run)
    _tp("build_program + nc.compile done")

    tw.join()
    in_maps = warm["maps"]
    th.join()
    _tp("threads joined")
    ss_lo, ss_hi = presim["r"]
    mid = 0.5 * (ss_lo + ss_hi)
    ss_lo = max(1e-4, ss_lo - 0.35 * (mid - ss_lo) - 0.05)
    ss_hi = ss_hi + 0.35 * (ss_hi - mid) + 0.05
    tau = float(np.asarray(inputs["tau01"]).reshape(-1)[0])
    gam = float(np.asarray(inputs["gamma1"]).reshape(-1)[0])
    poly, perr = _fit_sarg_poly(tau, gam, ss_lo, ss_hi)   # highest-first
    for m in in_maps:
        m["packs"][OFS_COEF:OFS_COEF + 6] = [
            ss_lo, ss_hi, poly[0], poly[1], poly[2], poly[3]]

    from concourse import bass_utils
    exec_ns = mean_ns = None
    _t0 = _time.perf_counter()
    try:
        outs = _run_fast(nc, in_maps, warm.get("pre"))
        runner = "fast"
        exec_ns = int(outs.pop("__exec_wall_s__", 0) * 1e9) or None
        ytg = outs["yt"].reshape(NCORES, NCLS, NT)
    except Exception:
        res = bass_utils.run_bass_kernel_spmd(
            nc, in_maps, core_ids=list(range(NCORES)))
        runner = "bass_utils"
        exec_ns, mean_ns = res.exec_time_ns, res.mean_exec_time_ns
        ytg = np.stack([np.asarray(res.results[c]["yt"])
                        for c in range(NCORES)])
    _run_wall = _time.perf_counter() - _t0

    ytg = ytg.astype(np.float32)[:, :, :t_run * BLOC]
    y = np.ascontiguousarray(
        ytg.reshape(NCORES, NCLS, t_run, BLOC).transpose(0, 3, 2, 1)
        .reshape(B, t_run, NCLS))
    if t_run < T:
        yf = np.zeros((B, T, NCLS), np.float32)
        yf[:, :t_run] = y
        y = yf
    _LAST_RUN.clear()
    _LAST_RUN.update(dict(exec_time_ns=exec_ns, mean_exec_time_ns=mean_ns,
                          run_wall_s=_run_wall, poly_err=perr,
                          ss_lo=ss_lo, ss_hi=ss_hi, runner=runner))
    return y



# revision 10
# speedup vs baseline: 1.4952x; 1.4952x over previous
"""Trainium2 Bass kernel for nn_DREAMAcousticNL (2-layer liquid-RNN over T=1000).

Strategy
--------
Key algebraic facts about the reference:
  * cell0's recurrent state h0 is dead code: `base_eff` (the signal fed to
    cell1 and to the output) depends only on x_t.  So only cell1's scan is
    sequential.
  * Everything that depends only on the inputs is precomputed as batched
    matmuls (phase A): be0 = clip_norm(x) @ B0.T, its norm xs1, be0' =
    be0/xs1, be1 = clip(be0') @ B1.T, plus the be1-part of the output head.
  * Per scan step t only:  m1 = h @ C1.T ; th = tanh(m1) ; err' = be0'-th ;
    ss = |err'|^2 ; s = sigmoid(P3(ss)) with P3 a cubic fitted to
    (min(sqrt(ss),4)-tau)/gamma on the presimulated ss range (tanh and
    sigmoid share one ACT table, so no table thrash) ; u = err' @ W1.T ;
    ih = 0.2 h + 0.6 be1 + (0.2 s xs1) u ; h' = h (1-g) + tanh(ih) g with
    g = s*sig(a1).
  * Head: y = h_seq @ head_w[:,:512].T + be1 @ head_w[:,512:].T + head_b.

Wall-clock structure (the graded metric): the bass_exec path recompiles
BIR->NEFF per run and re-uploads all inputs through the axon relay, so
program SIZE and WIRE BYTES/ARRAY COUNT dominate, not device time (~50ms).
Hence:
  * The scan runs as a tc.For_i HARDWARE loop (UNROLL steps/iteration) with
    ds() dynamic column offsets into SBUF history tensors (~1k instrs
    total vs ~55k fully unrolled -> seconds of neuronx-cc per run saved).
  * All large inputs ship as ONE bf16 pack + ONE small f32 pack per core
    (upload latency is per-array x per-shard); weights are converted to
    f32 in SBUF right after DMA, so compute precision is unchanged except
    for the bf16 rounding of the weights/feats themselves.
  * The surprise-poly coefficients and clamp range are RUNTIME inputs, so
    build_program is input-independent and the numpy presimulation that
    produces them runs on a thread concurrently with build+compile.
  * The output ships bf16 (halves the donated-zero upload + fetch).

Distribution: data-parallel over batch, B=16 -> 8 cores x 2 rows (SPMD).

Layout ("T-layout"): features on partitions.  Per-core tensors are
(128, 4, 2*T): partition p + chunk kc hold feature kc*128+p, free column
t*2+b.  Per-step matmuls use 128x128 stationary weight blocks
(lhsT = W.T block) with the (128,2) state slice as the moving operand, so
outputs stay feature-on-partition.  Norm reductions across partitions use a
ones-column matmul; per-row scalars are broadcast back to 128 partitions
with a rank-1 outer-product matmul.
"""

import os
import threading

import numpy as np
import ml_dtypes

B, T, MEL, HID, NCLS = 16, 1000, 80, 512, 64
NCORES = 8
BLOC = B // NCORES          # 2 batch rows per core
NT = T * BLOC               # free columns of history tensors
KC = HID // 128             # 4 feature chunks
NCH = 4                     # phase-A column chunks

# bf16 pack offsets (elements)
SZ_FEATS = MEL * NT
SZ_W = KC * 128 * HID
SZ_B0 = MEL * HID
SZ_WH = KC * 128 * NCLS
OF_FEATS = 0
OF_C1 = OF_FEATS + SZ_FEATS
OF_W1 = OF_C1 + SZ_W
OF_B1 = OF_W1 + SZ_W
OF_B0 = OF_B1 + SZ_W
OF_WH1 = OF_B0 + SZ_B0
OF_WH2 = OF_WH1 + SZ_WH
NB = OF_WH2 + SZ_WH
# f32 pack offsets
OFS_HEADB = 0
OFS_SIGA = OFS_HEADB + NCLS
OFS_COEF = OFS_SIGA + 128 * KC * BLOC
NS = OFS_COEF + 8

_LAST_RUN = {}
_TP = lambda msg: None   # timing probe, set by kernel() when KERNEL_TIMING=1


# ---------------------------------------------------------------- host math
def _np_phase_a(inputs):
    f32 = np.float32
    feats = inputs["feats"].astype(f32)
    ss0 = np.einsum("btm,btm->bt", feats, feats)
    xs0 = np.maximum(np.sqrt(ss0), 1e-6)[..., None].astype(f32)
    xn0 = np.clip(feats * (f32(1.0) / xs0), -1, 1).astype(f32)
    be0 = (xn0 @ inputs["B0"].astype(f32).T).astype(f32)
    ss1 = np.einsum("btm,btm->bt", be0, be0)
    xs1 = np.maximum(np.sqrt(ss1), 1e-6)[..., None].astype(f32)
    be0p = (be0 * (f32(1.0) / xs1)).astype(f32)
    xn1 = np.clip(be0p, -1, 1).astype(f32)
    be1 = (xn1 @ inputs["B1"].astype(f32).T).astype(f32)
    return be0p, xs1[..., 0], be1


def _np_ss_range(inputs, t_run):
    """Presimulate the scan in numpy to find the range of ss = |err'|^2."""
    f32 = np.float32
    be0p, xs1, be1 = _np_phase_a(inputs)
    C1 = inputs["C1"].astype(f32)
    W1 = inputs["W1"].astype(f32)
    a1 = inputs["a1"].astype(f32)
    tau = float(inputs["tau01"][0])
    gam = float(inputs["gamma1"][0])
    siga = (1 / (1 + np.exp(-a1))).astype(f32)
    h = np.zeros((B, HID), f32)
    lo, hi = np.inf, -np.inf
    for t in range(t_run):
        m1 = h @ C1.T
        err = be0p[:, t, :] - np.tanh(m1)
        ss = np.einsum("bh,bh->b", err, err)
        lo = min(lo, ss.min()); hi = max(hi, ss.max())
        rel = np.minimum(np.sqrt(ss), 4.0)
        s = 1 / (1 + np.exp(-(rel - tau) / gam))
        u = err @ W1.T
        ih = 0.2 * h + 0.6 * be1[:, t, :] + (0.2 * s * xs1[:, t])[:, None] * u
        h = h + (s[:, None] * siga[None, :]) * (np.tanh(ih) - h)
    return float(lo), float(hi)


def _fit_sarg_poly(tau, gam, lo, hi):
    """Cubic fit of (min(sqrt(x),4)-tau)/gam on [lo,hi]; coeffs highest-first."""
    xg = np.linspace(lo, hi, 2001)
    tgt = (np.minimum(np.sqrt(xg), 4.0) - tau) / gam
    ch = np.polynomial.chebyshev.Chebyshev.fit(xg, tgt, 3)
    co = np.polynomial.chebyshev.cheb2poly(ch.convert().coef)
    err = np.max(np.abs(np.polyval(co[::-1], xg) - tgt))
    return [float(c) for c in co[::-1]], float(err)


# ------------------------------------------------------------- bass program
def build_program(t_run, num_devices=NCORES):
    import concourse.bacc as bacc
    import concourse.bass as bass
    import concourse.mybir as mybir
    import concourse.tile as tile

    ds = bass.ds
    dt = mybir.dt
    f32 = dt.float32
    bf16 = dt.bfloat16
    HDT = bf16
    AF = mybir.ActivationFunctionType
    OP = mybir.AluOpType
    ntr = t_run * BLOC
    chw = ntr // NCH
    UNROLL = int(os.environ.get("KERNEL_UNROLL", "4"))
    STEP = UNROLL * BLOC
    assert ntr % NCH == 0 and ntr % STEP == 0

    nc = bacc.Bacc("TRN2", target_bir_lowering=False, debug=False,
                   num_devices=num_devices)

    packb = nc.dram_tensor("packb", (NB,), bf16, kind="ExternalInput").ap()
    packs = nc.dram_tensor("packs", (NS,), f32, kind="ExternalInput").ap()
    yt = nc.dram_tensor("yt", (NCLS, NT), bf16, kind="ExternalOutput").ap()

    def seg_b(ofs, sz):
        return packb[ofs:ofs + sz]

    with tile.TileContext(nc) as tc:
        with (
            tc.tile_pool(name="const", bufs=1) as cpool,
            tc.tile_pool(name="hist", bufs=1) as hpool,
            tc.tile_pool(name="pha", bufs=2) as apool,
            tc.tile_pool(name="scan", bufs=2) as spool,
            tc.tile_pool(name="pp", bufs=2, space="PSUM") as pp,
        ):
            # ---- unpack + bf16->f32 convert of weights/feats
            sb_feats = cpool.tile([MEL, ntr], f32)
            sb_c1 = cpool.tile([128, KC, HID], f32)
            sb_w1 = cpool.tile([128, KC, HID], f32)
            sb_b0 = cpool.tile([MEL, HID], f32)
            sb_b1 = cpool.tile([128, KC, HID], f32)
            sb_wh1 = cpool.tile([128, KC, NCLS], f32)
            sb_wh2 = cpool.tile([128, KC, NCLS], f32)
            sb_headb = cpool.tile([NCLS, 1], f32)
            sb_siga = cpool.tile([128, KC, BLOC], f32)
            sb_cf = cpool.tile([1, 8], f32)
            sb_ones = cpool.tile([128, 1], f32)       # column of ones
            sb_onesT = cpool.tile([1, 128], f32)      # row of ones

            def load_cvt(dst, ofs, sz, pat, tag, **kw):
                stg = apool.tile(list(dst.shape), bf16, tag=tag)
                nc.sync.dma_start(stg[:], seg_b(ofs, sz).rearrange(pat, **kw))
                nc.vector.tensor_copy(dst[:], stg[:])

            load_cvt(sb_feats, OF_FEATS, SZ_FEATS, "(m n) -> m n", "lf",
                     m=MEL)
            load_cvt(sb_c1, OF_C1, SZ_W, "(k p n) -> p k n", "l0", k=KC,
                     p=128)
            load_cvt(sb_w1, OF_W1, SZ_W, "(k p n) -> p k n", "l1", k=KC,
                     p=128)
            load_cvt(sb_b1, OF_B1, SZ_W, "(k p n) -> p k n", "l0", k=KC,
                     p=128)
            load_cvt(sb_b0, OF_B0, SZ_B0, "(m n) -> m n", "l1", m=MEL)
            load_cvt(sb_wh1, OF_WH1, SZ_WH, "(k p n) -> p k n", "lw",
                     k=KC, p=128)
            load_cvt(sb_wh2, OF_WH2, SZ_WH, "(k p n) -> p k n", "lw",
                     k=KC, p=128)
            nc.sync.dma_start(
                sb_headb[:],
                packs[OFS_HEADB:OFS_HEADB + NCLS].rearrange("(m n) -> m n",
                                                            m=NCLS))
            nc.sync.dma_start(
                sb_siga[:],
                packs[OFS_SIGA:OFS_SIGA + 128 * KC * BLOC].rearrange(
                    "(p k b) -> p k b", p=128, k=KC))
            nc.sync.dma_start(
                sb_cf[:],
                packs[OFS_COEF:OFS_COEF + 8].rearrange("(m n) -> m n", m=1))
            nc.vector.memset(sb_ones[:], 1.0)
            nc.vector.memset(sb_onesT[:], 1.0)

            # ---- persistent per-core state
            sb_be0f = hpool.tile([128, KC, ntr], f32)   # be0 then (in-place) xn1
            sb_be0p = hpool.tile([128, KC, ntr], HDT)   # be0/xs1 history
            sb_be1s = hpool.tile([128, KC, ntr], HDT)   # 0.6*be1 history
            # h history, 1 zero slot in front: column o = t*BLOC is h BEFORE
            # step t; the scan writes h_t at o+BLOC
            sb_hs = hpool.tile([128, KC, ntr + BLOC], f32)
            sb_xs1s = hpool.tile([1, ntr], f32)         # 0.2*xs1
            sb_yt = hpool.tile([NCLS, ntr], f32)

            # ================= phase A =================
            for ch in range(NCH):
                R = slice(ch * chw, (ch + 1) * chw)
                sq = apool.tile([128, chw], f32, tag="sqA")
                p_ss = pp.tile([1, chw], f32, tag="p2")
                p_bc = pp.tile([128, chw], f32, tag="p3")
                rowA = apool.tile([1, chw], f32, tag="rowA")
                rowB = apool.tile([1, chw], f32, tag="rowB")

                # |x|^2 over 80 input dims
                nc.vector.tensor_tensor(sq[:MEL, :], sb_feats[:, R],
                                        sb_feats[:, R], OP.mult)
                nc.tensor.matmul(p_ss[:], sb_ones[:MEL, :], sq[:MEL, :],
                                 start=True, stop=True)
                nc.scalar.activation(rowA[:], p_ss[:], AF.Sqrt)
                nc.vector.tensor_scalar_max(rowA[:], rowA[:], 1e-6)
                nc.vector.reciprocal(rowB[:], rowA[:])
                nc.tensor.matmul(p_bc[:MEL, :], sb_onesT[:, :MEL], rowB[:],
                                 start=True, stop=True)
                # xn0 = clip(x/|x|) in place
                nc.vector.tensor_tensor(sb_feats[:, R], sb_feats[:, R],
                                        p_bc[:MEL, :], OP.mult)
                nc.vector.tensor_scalar(sb_feats[:, R], sb_feats[:, R],
                                        -1.0, 1.0, OP.max, OP.min)
                # be0 = xn0 @ B0.T
                for mc in range(KC):
                    p_be = pp.tile([128, chw], f32, tag="p0")
                    nc.tensor.matmul(p_be[:], sb_b0[:, mc * 128:(mc + 1) * 128],
                                     sb_feats[:, R], start=True, stop=True)
                    nc.vector.tensor_copy(sb_be0f[:, mc, R], p_be[:])
                # |be0|^2 over 512
                for kc in range(KC):
                    nc.vector.tensor_tensor(sq[:], sb_be0f[:, kc, R],
                                            sb_be0f[:, kc, R], OP.mult)
                    nc.tensor.matmul(p_ss[:], sb_ones[:], sq[:],
                                     start=(kc == 0), stop=(kc == KC - 1))
                nc.scalar.activation(rowA[:], p_ss[:], AF.Sqrt)
                nc.vector.tensor_scalar_max(rowA[:], rowA[:], 1e-6)
                nc.vector.tensor_scalar_mul(sb_xs1s[:, R], rowA[:], 0.2)
                nc.vector.reciprocal(rowB[:], rowA[:])
                nc.tensor.matmul(p_bc[:], sb_onesT[:], rowB[:],
                                 start=True, stop=True)
                for kc in range(KC):
                    # be0' = be0/xs1 (bf16 history), xn1 = clip(be0') in place
                    nc.vector.tensor_tensor(sb_be0p[:, kc, R], sb_be0f[:, kc, R],
                                            p_bc[:], OP.mult)
                    nc.vector.tensor_tensor(sb_be0f[:, kc, R], sb_be0f[:, kc, R],
                                            p_bc[:], OP.mult)
                    nc.vector.tensor_scalar(sb_be0f[:, kc, R], sb_be0f[:, kc, R],
                                            -1.0, 1.0, OP.max, OP.min)
                # be1 = xn1 @ B1.T ; also y2 = Wh2 @ be1T accumulated
                p_y2 = pp.tile([NCLS, chw], f32, tag="p1")
                tmp_be1 = apool.tile([128, chw], f32, tag="tbe1A")
                for mc in range(KC):
                    p_be1 = pp.tile([128, chw], f32, tag="p0")
                    for kc in range(KC):
                        nc.tensor.matmul(
                            p_be1[:],
                            sb_b1[:, kc, mc * 128:(mc + 1) * 128],
                            sb_be0f[:, kc, R],
                            start=(kc == 0), stop=(kc == KC - 1))
                    nc.vector.tensor_scalar_mul(sb_be1s[:, mc, R], p_be1[:], 0.6)
                    nc.scalar.copy(tmp_be1[:], p_be1[:])
                    nc.tensor.matmul(p_y2[:], sb_wh2[:, mc, :], tmp_be1[:],
                                     start=(mc == 0), stop=(mc == KC - 1))
                nc.vector.tensor_copy(sb_yt[:, R], p_y2[:])

            # zero h slot 0
            nc.vector.memset(sb_hs[:, :, 0:BLOC], 0.0)

            # scheduler fence: keep phase-A ACT (sqrt set) strictly before the
            # scan's tanh/sigmoid stream to avoid activation-table thrash
            tc.no_sync_barrier()

            # ================= phase B: the scan (hardware loop) ============
            with tc.For_i(0, ntr, STEP) as iv:
                for k in range(UNROLL):
                    o = iv + k * BLOC

                    pm1 = pp.tile([128, KC * BLOC], f32, tag="p0")
                    pu = pp.tile([128, KC * BLOC], f32, tag="p1")
                    pss = pp.tile([1, KC * BLOC], f32, tag="p2")
                    pbc = pp.tile([128, 2 * BLOC], f32, tag="p3")
                    th = spool.tile([128, KC, BLOC], HDT, tag="th")
                    err = spool.tile([128, KC, BLOC], f32, tag="err")
                    sqt = spool.tile([128, KC * BLOC], f32, tag="sqt")
                    ssb = spool.tile([1, BLOC], f32, tag="ssb")
                    acc = spool.tile([1, BLOC], f32, tag="acc")
                    accB = spool.tile([1, BLOC], f32, tag="accB")
                    row4 = spool.tile([1, 2 * BLOC], f32, tag="row4")
                    gt = spool.tile([128, KC, BLOC], f32, tag="gt")
                    gtm = spool.tile([128, KC, BLOC], f32, tag="gtm")
                    p1 = spool.tile([128, KC, BLOC], f32, tag="p1s")
                    vt = spool.tile([128, KC, BLOC], f32, tag="vt")
                    wt = spool.tile([128, KC, BLOC], f32, tag="wt")
                    ih = spool.tile([128, KC, BLOC], f32, tag="ih")
                    th2 = spool.tile([128, KC, BLOC], f32, tag="th2")
                    q1 = spool.tile([128, KC, BLOC], f32, tag="q1")

                    # m1.T = C1 @ h.T (16 blocks, moving operand = h slice)
                    for mc in range(KC):
                        for kc in range(KC):
                            nc.tensor.matmul(
                                pm1[:, mc * BLOC:(mc + 1) * BLOC],
                                sb_c1[:, kc, mc * 128:(mc + 1) * 128],
                                sb_hs[:, kc, ds(o, BLOC)],
                                start=(kc == 0), stop=(kc == KC - 1))
                    nc.scalar.activation(th[:], pm1[:], AF.Tanh)
                    nc.vector.tensor_tensor(err[:], sb_be0p[:, :, ds(o, BLOC)],
                                            th[:], OP.subtract)
                    nc.vector.tensor_tensor(sqt[:], err[:], err[:], OP.mult)
                    nc.tensor.matmul(pss[:], sb_ones[:], sqt[:],
                                     start=True, stop=True)
                    # ss per row: sum the 4 chunk partials (cols kc-major)
                    nc.vector.tensor_reduce(
                        ssb[:], pss.rearrange("p (k b) -> p b k", k=KC),
                        mybir.AxisListType.X, OP.add)
                    # s = sigmoid(P3(clamp(ss))), P3 ~ (min(sqrt,4)-tau)/gam
                    nc.vector.tensor_scalar(ssb[:], ssb[:], sb_cf[:, 0:1],
                                            sb_cf[:, 1:2], OP.max, OP.min)
                    nc.vector.tensor_scalar(acc[:], ssb[:], sb_cf[:, 2:3],
                                            sb_cf[:, 3:4], OP.mult, OP.add)
                    nc.vector.tensor_tensor(accB[:], acc[:], ssb[:], OP.mult)
                    nc.vector.tensor_scalar_add(accB[:], accB[:], sb_cf[:, 4:5])
                    nc.vector.tensor_tensor(acc[:], accB[:], ssb[:], OP.mult)
                    nc.scalar.activation(row4[:, 0:BLOC], acc[:], AF.Sigmoid,
                                         bias=sb_cf[:, 5:6])
                    # c = 0.2*s*xs1
                    nc.vector.tensor_tensor(row4[:, BLOC:2 * BLOC],
                                            row4[:, 0:BLOC],
                                            sb_xs1s[:, ds(o, BLOC)], OP.mult)
                    # u.T = W1 @ err'.T
                    for mc in range(KC):
                        for kc in range(KC):
                            nc.tensor.matmul(
                                pu[:, mc * BLOC:(mc + 1) * BLOC],
                                sb_w1[:, kc, mc * 128:(mc + 1) * 128],
                                err[:, kc, :],
                                start=(kc == 0), stop=(kc == KC - 1))
                    # broadcast [s0,s1,c0,c1] to all partitions
                    nc.tensor.matmul(pbc[:], sb_onesT[:], row4[:],
                                     start=True, stop=True)
                    puv = pu.rearrange("p (k b) -> p k b", k=KC)
                    # g = s*sig(a1); gm = 1-g; both via stride-0 kc-broadcast
                    nc.vector.tensor_tensor(
                        gt[:], sb_siga[:],
                        pbc[:, None, 0:BLOC].broadcast_to([128, KC, BLOC]),
                        OP.mult)
                    nc.vector.tensor_scalar(gtm[:], gt[:], -1.0, 1.0,
                                            OP.mult, OP.add)
                    nc.vector.tensor_tensor(p1[:], sb_hs[:, :, ds(o, BLOC)],
                                            gtm[:], OP.mult)
                    # vt = c*u: u is in PSUM, so the c broadcast must come
                    # from SBUF (TensorTensor reads at most one PSUM input)
                    sbc2 = spool.tile([128, BLOC], f32, tag="sbc2")
                    nc.vector.tensor_copy(sbc2[:], pbc[:, BLOC:2 * BLOC])
                    nc.vector.tensor_tensor(
                        vt[:], puv,
                        sbc2[:, None, :].broadcast_to([128, KC, BLOC]),
                        OP.mult)
                    # ih = 0.2 h + 0.6 be1 + c*u
                    nc.vector.scalar_tensor_tensor(
                        wt[:], sb_hs[:, :, ds(o, BLOC)], 0.2,
                        sb_be1s[:, :, ds(o, BLOC)], OP.mult, OP.add)
                    nc.vector.tensor_tensor(ih[:], wt[:], vt[:], OP.add)
                    nc.scalar.activation(th2[:], ih[:], AF.Tanh)
                    # h' = h*(1-g) + tanh(ih)*g
                    nc.vector.tensor_tensor(q1[:], th2[:], gt[:], OP.mult)
                    nc.vector.tensor_tensor(sb_hs[:, :, ds(o + BLOC, BLOC)],
                                            p1[:], q1[:], OP.add)

            tc.no_sync_barrier()

            # ================= phase C: head =================
            for ch in range(NCH):
                R = slice(ch * chw, (ch + 1) * chw)
                Rh = slice(BLOC + ch * chw, BLOC + (ch + 1) * chw)
                p_y1 = pp.tile([NCLS, chw], f32, tag="p1")
                ytb = apool.tile([NCLS, chw], bf16, tag="ytb")
                for kc in range(KC):
                    nc.tensor.matmul(p_y1[:], sb_wh1[:, kc, :],
                                     sb_hs[:, kc, Rh],
                                     start=(kc == 0), stop=(kc == KC - 1))
                nc.vector.tensor_tensor(sb_yt[:, R], sb_yt[:, R], p_y1[:],
                                        OP.add)
                nc.vector.tensor_scalar(ytb[:], sb_yt[:, R],
                                        sb_headb[:], None, OP.add)
                nc.sync.dma_start(yt[:, R], ytb[:])

    nc.compile()
    return nc


# ------------------------------------------------------------- input prep
def _prep_inputs(inputs):
    """Build the per-core packed input maps (host-side layout only)."""
    f32c = lambda k: np.ascontiguousarray(inputs[k], dtype=np.float32)
    bf16 = ml_dtypes.bfloat16

    C1T = np.ascontiguousarray(f32c("C1").T)            # (512,512) [k, m]
    W1T = np.ascontiguousarray(f32c("W1").T)
    B1T = np.ascontiguousarray(f32c("B1").T)
    B0T = np.ascontiguousarray(f32c("B0").T)            # (80,512)
    head_w = f32c("head_w")
    Wh1T = np.ascontiguousarray(head_w[:, :HID].T)      # (512,64)
    Wh2T = np.ascontiguousarray(head_w[:, HID:].T)
    a1 = f32c("a1")
    siga = (1.0 / (1.0 + np.exp(-a1))).astype(np.float32)   # (512,)
    # (128, KC, BLOC): siga8[p, kc, b] = siga[kc*128+p]
    siga8 = np.repeat(siga.reshape(KC, 128).T[:, :, None], BLOC, axis=2)

    wseg = np.empty(NB - SZ_FEATS, bf16)

    def put(ofs, arr):
        fl = np.asarray(arr, np.float32).reshape(-1)
        wseg[ofs - SZ_FEATS:ofs - SZ_FEATS + fl.size] = fl.astype(bf16)

    put(OF_C1, C1T)       # flat (k p n): C1T.reshape(KC,128,512) row-major
    put(OF_W1, W1T)
    put(OF_B1, B1T)
    put(OF_B0, B0T)
    put(OF_WH1, Wh1T)
    put(OF_WH2, Wh2T)

    packs = np.zeros(NS, np.float32)
    packs[OFS_HEADB:OFS_HEADB + NCLS] = f32c("head_b")
    packs[OFS_SIGA:OFS_SIGA + 128 * KC * BLOC] = siga8.reshape(-1)
    # packs[OFS_COEF:] filled later (after the presim joins)

    feats = f32c("feats")                                # (16,1000,80)
    in_maps = []
    for c in range(NCORES):
        fl = feats[c * BLOC:(c + 1) * BLOC]              # (2,1000,80)
        # featsT[m, t*2+b] = feats[b, t, m]
        ftT = fl.transpose(2, 1, 0).reshape(-1).astype(bf16)
        packb = np.concatenate([ftT, wseg])
        in_maps.append({"packb": packb, "packs": packs})
    return in_maps


# ------------------------------------------------------------- fast runner
def _run_fast(nc, in_maps, pre):
    """shard_map runner for the prebuilt Bass module.  Vs the stock
    bass_utils path it (a) consumes inputs ALREADY device_put asynchronously
    during build/compile (`pre`), so the 17MB upload overlaps the host-side
    compile instead of serializing after it, and (b) gathers the sharded
    output once instead of once per core.  Any failure falls back to
    bass_utils.run_bass_kernel_spmd in kernel()."""
    import jax
    import concourse.mybir as mybir
    from concourse.bass2jax import (_bass_exec_p, install_neuronx_cc_hook,
                                    partition_id_tensor)
    from jax.sharding import Mesh, PartitionSpec
    try:
        from jax.experimental.shard_map import shard_map
    except ImportError:
        shard_map = jax.shard_map

    install_neuronx_cc_hook()
    assert nc.dbg_addr is None
    n_cores = len(in_maps)

    in_names, out_names, out_avals, zero_shapes = [], [], [], []
    for alloc in nc.m.functions[0].allocations:
        if not isinstance(alloc, mybir.MemoryLocationSet):
            continue
        name = alloc.memorylocations[0].name
        if alloc.kind == "ExternalInput":
            in_names.append(name)
        elif alloc.kind == "ExternalOutput":
            shape = tuple(alloc.tensor_shape)
            dtype = mybir.dt.np(alloc.dtype)
            out_names.append(name)
            out_avals.append(jax.core.ShapedArray(shape, dtype))
            zero_shapes.append((shape, dtype))
    partition_name = (nc.partition_id_tensor.name
                      if nc.partition_id_tensor else None)
    if partition_name is not None:
        in_names.remove(partition_name)
    n_params = len(in_names)
    n_outs = len(out_names)
    all_names = in_names + out_names + (
        [partition_name] if partition_name else [])
    donate = tuple(range(n_params, n_params + n_outs))

    def _body(*args):
        operands = list(args)
        if partition_name is not None:
            operands.append(partition_id_tensor())
        outs = _bass_exec_p.bind(
            *operands, out_avals=tuple(out_avals), in_names=tuple(all_names),
            out_names=tuple(out_names), lowering_input_output_aliases=(),
            sim_require_finite=True, sim_require_nnan=True, nc=nc)
        return tuple(outs)

    devices = jax.devices()[:n_cores]
    mesh = Mesh(np.asarray(devices), ("core",))
    in_specs = (PartitionSpec("core"),) * (n_params + n_outs)
    out_specs = (PartitionSpec("core"),) * n_outs
    sharded = jax.jit(
        shard_map(_body, mesh=mesh, in_specs=in_specs, out_specs=out_specs,
                  check_rep=False),
        donate_argnums=donate, keep_unused=True)

    pre = pre or {}
    args = []
    for name in in_names:
        if name in pre:
            args.append(pre[name])
        else:
            args.append(np.concatenate(
                [np.asarray(in_maps[c][name]) for c in range(n_cores)],
                axis=0))
    for name, (shape, dtype) in zip(out_names, zero_shapes):
        zkey = f"__zeros_{name}__"
        gshape = (n_cores * shape[0], *shape[1:])
        if (zkey in pre and tuple(pre[zkey].shape) == gshape
                and pre[zkey].dtype == dtype):
            args.append(pre[zkey])
        else:
            args.append(np.zeros(gshape, dtype))
    _TP("run_fast: args ready")
    compiled = sharded.lower(*args).compile()
    _TP("run_fast: jit lower+compile done")
    import time as _time
    _t0 = _time.perf_counter()
    out_arrs = compiled(*args)
    _TP("run_fast: dispatch returned")
    res = {name: np.asarray(out_arrs[i]) for i, name in enumerate(out_names)}
    # dispatch→fetch wall: device execution + result download (upper bound
    # on HW exec; excludes host-side jit/NEFF compilation, which is not
    # execution).  Recorded for kernel() to surface as exec_time_ns.
    # (A per-shard parallel fetch was tried and measured SLOWER: the global
    # asarray gather rides along with execution completion, while separate
    # shard pulls add ~0.36s of extra relay round trips.)
    res["__exec_wall_s__"] = _time.perf_counter() - _t0
    _TP("run_fast: outputs fetched")
    return res


def kernel(**inputs):
    import time as _time
    import sys as _sys

    _tstart = _time.perf_counter()
    _dbg = os.environ.get("KERNEL_TIMING")

    def _tp(msg):
        if _dbg:
            print(f"[ktime {_time.perf_counter() - _tstart:8.2f}s] {msg}",
                  file=_sys.stderr, flush=True)

    # inputs may arrive as jax arrays; all host math below assumes numpy
    inputs = {k: np.asarray(v) for k, v in inputs.items()}
    global _TP
    _TP = _tp
    _tp("inputs to numpy")

    t_run = int(os.environ.get("KERNEL_T", T))

    # presimulation (for the surprise-poly fit range) runs concurrently
    # with build_program: its results are runtime inputs, not constants.
    # A second thread forces jax/PJRT (axon) init, packs the inputs, and
    # starts the async upload of the big per-core pack + donated output
    # zeros, so both overlap the bass build + jit compile.
    presim = {}
    def _presim():
        presim["r"] = _np_ss_range(inputs, t_run)
        _tp("presim done")
    th = threading.Thread(target=_presim)
    th.start()
    warm = {}
    def _warm():
        import jax
        devs = jax.devices()
        _tp("jax.devices ready")
        warm["maps"] = _prep_inputs(inputs)
        _tp("prep_inputs done")
        try:
            from jax.sharding import Mesh, PartitionSpec, NamedSharding
            mesh = Mesh(np.asarray(devs[:NCORES]), ("core",))
            sh = NamedSharding(mesh, PartitionSpec("core"))
            cb = np.concatenate([m["packb"] for m in warm["maps"]])
            z = np.zeros((NCORES * NCLS, NT), ml_dtypes.bfloat16)
            warm["pre"] = {"packb": jax.device_put(cb, sh),
                           "__zeros_yt__": jax.device_put(z, sh)}
            _tp("device_put issued")
        except Exception:
            warm["pre"] = None
    tw = threading.Thread(target=_warm)
    tw.start()

    nc = build_program(t_run)
    _tp("build_program + nc.compile done")

    tw.join()
    in_maps = warm["maps"]
    th.join()
    _tp("threads joined")
    ss_lo, ss_hi = presim["r"]
    mid = 0.5 * (ss_lo + ss_hi)
    ss_lo = max(1e-4, ss_lo - 0.35 * (mid - ss_lo) - 0.05)
    ss_hi = ss_hi + 0.35 * (ss_hi - mid) + 0.05
    tau = float(np.asarray(inputs["tau01"]).reshape(-1)[0])
    gam = float(np.asarray(inputs["gamma1"]).reshape(-1)[0])
    poly, perr = _fit_sarg_poly(tau, gam, ss_lo, ss_hi)   # highest-first
    for m in in_maps:
        m["packs"][OFS_COEF:OFS_COEF + 6] = [
            ss_lo, ss_hi, poly[0], poly[1], poly[2], poly[3]]

    from concourse import bass_utils
    exec_ns = mean_ns = None
    _t0 = _time.perf_counter()
    try:
        outs = _run_fast(nc, in_maps, warm.get("pre"))
        runner = "fast"
        exec_ns = int(outs.pop("__exec_wall_s__", 0) * 1e9) or None
        ytg = outs["yt"].reshape(NCORES, NCLS, NT)
    except Exception:
        res = bass_utils.run_bass_kernel_spmd(
            nc, in_maps, core_ids=list(range(NCORES)))
        runner = "bass_utils"
        exec_ns, mean_ns = res.exec_time_ns, res.mean_exec_time_ns
        ytg = np.stack([np.asarray(res.results[c]["yt"])
                        for c in range(NCORES)])
    _run_wall = _time.perf_counter() - _t0

    ytg = ytg.astype(np.float32)[:, :, :t_run * BLOC]
    y = np.ascontiguousarray(
        ytg.reshape(NCORES, NCLS, t_run, BLOC).transpose(0, 3, 2, 1)
        .reshape(B, t_run, NCLS))
    if t_run < T:
        yf = np.zeros((B, T, NCLS), np.float32)
        yf[:, :t_run] = y
        y = yf
    _LAST_RUN.clear()
    _LAST_RUN.update(dict(exec_time_ns=exec_ns, mean_exec_time_ns=mean_ns,
                          run_wall_s=_run_wall, poly_err=perr,
                          ss_lo=ss_lo, ss_hi=ss_hi, runner=runner))
    return y

